# revision 2
# baseline (speedup 1.0000x reference)
"""Trainium2 Bass kernel for a dense transformer block (attention + ReLU FFN).

Reference computation (B=4, T=2048, C=1024, H=16, D=64):
    q,k,v = per-head projections of x;  causal softmax(q k^T / sqrt(C)) v;
    concat heads;  y = relu(out @ Wf.T + bf)

Sharding over 8 NeuronCores: core (2b+p) handles batch b with heads
[8p, 8p+8).  Attention runs causally over the full T on each core.  Pair
AllGathers (cores 2b/2b+1) share the attention outputs, and each core
runs the FFN for all 2048 tokens over its own half of the output
channels (the channel split is carried entirely by per-core input data -
every core executes an identical NEFF).

Layouts: scores are computed transposed ([s, t], keys on partitions) so
the exp() output feeds the AV matmul directly; V carries an appended
ones-column so row 64 of the AV accumulator is the softmax denominator;
causal masking is a -1e4 rank-128 matmul accumulated into the diagonal
score tile before exp. Compute dtype bf16 with fp32 PSUM accumulation.

Scheduling: a 48-matmul warmup burst on the (tiny, loaded-first) mask
constants heats the PE HAM clock-gate while the input DMAs stream;
projection/FFN matmul chunks are drained from filler queues inside the
attention j-loops so the PE never idles long enough to re-throttle; the
AllGathers run pair-wise and are emitted as soon as their two heads are
staged (the z-broadcast DMAs ride the Sync queue so the GpSimd queue
holds only collective triggers); the second-half FFN accumulates in
three phases so only the ci{3,7} matmuls depend on the last AllGather.
"""

import os
import sys

from collections import deque

import numpy as np
import ml_dtypes

for _p in ("/opt/trn_rl_repo", "/root/.axon_site/_ro/trn_rl_repo"):
    if os.path.isdir(_p) and _p not in sys.path:
        sys.path.append(_p)

B, T, C, H, D = 4, 2048, 1024, 16, 64
P = 128           # partitions
NCT = C // P      # 8 c-tiles
NTT = T // P      # 16 s/t-tiles
HPC = H // 2      # 8 heads per core
THALF = T // 2    # tokens per AllGather half
COH = C // 2      # output channels per core in the FFN
SCALE = float(C) ** -0.5
WARM_N = 48       # PE warmup matmuls (heats the HAM clock gate)

bf16 = ml_dtypes.bfloat16

_CACHE = {}


def build_nc():
    import concourse.bass as bass
    import concourse.tile as tile
    from concourse import bacc, mybir

    f32 = mybir.dt.float32
    b16 = mybir.dt.bfloat16
    EXP = mybir.ActivationFunctionType.Exp

    nc = bacc.Bacc("TRN2", target_bir_lowering=False, debug=False, num_devices=8)

    xT = nc.dram_tensor("xT", [C, T], b16, kind="ExternalInput").ap()
    wq = nc.dram_tensor("wq", [C, HPC * D], b16, kind="ExternalInput").ap()
    wk = nc.dram_tensor("wk", [C, HPC * D], b16, kind="ExternalInput").ap()
    wv = nc.dram_tensor("wv", [C, HPC * D], b16, kind="ExternalInput").ap()
    wfT = nc.dram_tensor("wfT", [C, COH], b16, kind="ExternalInput").ap()
    mey = nc.dram_tensor("mey", [P, P], b16, kind="ExternalInput").ap()
    mls_ = nc.dram_tensor("mls", [P, P], b16, kind="ExternalInput").ap()
    biasb = nc.dram_tensor("biasb", [P, COH], f32, kind="ExternalInput").ap()
    y = nc.dram_tensor("y", [T, COH], f32, kind="ExternalOutput").ap()
    # warmup sink: ExternalOutput so the warmup matmuls can't be DCE'd
    wsink = nc.dram_tensor("wsink", [P, P], f32, kind="ExternalOutput").ap()

    with tile.TileContext(nc) as tc, \
            tc.tile_pool(name="consts", bufs=1) as consts, \
            tc.tile_pool(name="dram", bufs=1, space="DRAM") as dram, \
            tc.tile_pool(name="sc_ps", bufs=2, space="PSUM") as sc_pool, \
            tc.tile_pool(name="av_ps", bufs=1, space="PSUM") as av_pool, \
            tc.tile_pool(name="flex_ps", bufs=2, space="PSUM") as flex_pool, \
            tc.tile_pool(name="wt", bufs=3) as wt_pool, \
            tc.tile_pool(name="norm", bufs=2) as norm_pool, \
            tc.tile_pool(name="yout", bufs=3) as y_pool:

        xT_sb = consts.tile([P, NCT, T], b16)
        wq_sb = consts.tile([P, NCT, HPC * D], b16)
        wk_sb = consts.tile([P, NCT, HPC * D], b16)
        wv_sb = consts.tile([P, NCT, HPC * D], b16)
        wfT_sb = consts.tile([P, NCT, COH], b16)
        mey_sb = consts.tile([P, P], b16)
        mls_sb = consts.tile([P, P], b16)
        biasb_sb = consts.tile([P, COH], f32)
        qT_sb = consts.tile([P, HPC // 2, T], b16)
        kT_sb = consts.tile([P, HPC // 2, T], b16)
        v_sb = consts.tile([P, NTT, HPC, D + 1], b16)
        ccout_sb = consts.tile([P, 2, NCT, THALF], b16)

        cc_in = [dram.tile([HPC * D, THALF], b16, name=f"cc_in{i}") for i in (0, 1)]
        cc_out = [[dram.tile([C // 4, THALF], b16, name=f"cc_out{th}_{p}")
                   for p in range(4)] for th in (0, 1)]

        # ---- constant loads, ordered so the warmup (mey/mls) and the first
        # QK projection chunks can start as early as possible ----------------
        nc.sync.dma_start(out=mey_sb, in_=mey)
        nc.sync.dma_start(out=mls_sb, in_=mls_)
        xT_r = xT.rearrange("(ct p) t -> ct p t", p=P)
        wq_r = wq.rearrange("(ct p) m -> ct p m", p=P)
        wk_r = wk.rearrange("(ct p) m -> ct p m", p=P)
        wv_r = wv.rearrange("(ct p) m -> ct p m", p=P)
        for ct in range(NCT):
            nc.sync.dma_start(out=wq_sb[:, ct, :], in_=wq_r[ct])
            nc.sync.dma_start(out=xT_sb[:, ct, 0:512], in_=xT_r[ct][:, 0:512])
        for ct in range(NCT):
            nc.sync.dma_start(out=wk_sb[:, ct, :], in_=wk_r[ct])
            nc.sync.dma_start(out=xT_sb[:, ct, 512:THALF],
                              in_=xT_r[ct][:, 512:THALF])
        for ct in range(NCT):
            nc.sync.dma_start(out=wv_sb[:, ct, :], in_=wv_r[ct])
        for ct in range(NCT):
            nc.sync.dma_start(out=xT_sb[:, ct, THALF:THALF + 512],
                              in_=xT_r[ct][:, THALF:THALF + 512])
            nc.sync.dma_start(out=xT_sb[:, ct, THALF + 512:T],
                              in_=xT_r[ct][:, THALF + 512:T])
        wfT_r = wfT.rearrange("(ct p) co -> ct p co", p=P)
        for ct in range(NCT):
            nc.sync.dma_start(out=wfT_sb[:, ct, :], in_=wfT_r[ct])
        nc.sync.dma_start(out=biasb_sb, in_=biasb)
        nc.vector.memset(v_sb[:, :, :, D:D + 1], 1.0)

        # ---- PE warmup: dense matmul burst on the mask constants while the
        # big input DMAs stream; keeps the HAM gate at 8/8 for the real work.
        with nc.named_scope("warmup"):
            wps = flex_pool.tile([P, P], f32, tag="flex", name="warmps")
            for i in range(WARM_N):
                nc.tensor.matmul(wps, lhsT=mey_sb, rhs=mls_sb,
                                 start=(i == 0), stop=(i == WARM_N - 1))
            wsb = y_pool.tile([P, P], f32, tag="y", name="warmsb")
            nc.vector.tensor_copy(out=wsb, in_=wps)
            nc.sync.dma_start(out=wsink, in_=wsb)

        # ---- emission helpers ----------------------------------------------
        def v_proj(st):
          with nc.named_scope("vproj"):
            ps = flex_pool.tile([P, 512], f32, tag="flex", name=f"vps{st}")
            for ct in range(NCT):
                nc.tensor.matmul(
                    ps, lhsT=xT_sb[:, ct, P * st:P * (st + 1)],
                    rhs=wv_sb[:, ct, :],
                    start=(ct == 0), stop=(ct == NCT - 1))
            nc.vector.tensor_copy(out=v_sb[:, st, :, 0:D],
                                  in_=ps.rearrange("p (h d) -> p h d", d=D))

        def qk_chunk(hp, i):
          with nc.named_scope("qkproj"):
            dst, w_t = ((qT_sb, wq_sb), (kT_sb, wk_sb))[i // 4]
            g = i % 4
            ps = flex_pool.tile([P, 512], f32, tag="flex", name=f"qkps{hp}_{i}")
            for ct in range(NCT):
                nc.tensor.matmul(
                    ps, lhsT=w_t[:, ct, hp * P:(hp + 1) * P],
                    rhs=xT_sb[:, ct, 512 * g:512 * (g + 1)],
                    start=(ct == 0), stop=(ct == NCT - 1))
            nc.vector.tensor_copy(out=dst[:, hp, 512 * g:512 * (g + 1)], in_=ps)

        def attn_unit(h, th, mid=None):
          with nc.named_scope(f"attn{th}_{h}"):
            hp, qh = divmod(h, 2)
            base = 64 * qh
            t0 = THALF * th
            av = av_pool.tile([P, THALF], f32, tag="av", name=f"av{h}_{th}")
            jmax = 8 * th + 8
            last_j = {0: 8 * th + 3, 1: jmax - 1}
            pend = None  # (j, pieces, wt) awaiting its AV emission

            def emit_av(ent):
                j, pieces, wt = ent
                for (o, e) in pieces:
                    region = 0 if o < 512 else 1
                    nc.tensor.matmul(
                        av[0:D + 1, o:e], lhsT=v_sb[:, j, h, :], rhs=wt[:, o:e],
                        start=(j == 0), stop=(j == last_j[region]))

            for j in range(jmax):
                off = max(0, P * j - t0)
                diag = P * j >= t0
                pieces = [(off, 512), (512, 1024)] if off < 512 \
                    else [(off, 1024)]
                sc = sc_pool.tile([P, THALF], f32, tag="sc", name=f"sc{h}_{th}_{j}")
                for pi, (o, e) in enumerate(pieces):
                    nc.tensor.matmul(
                        sc[:, o:e],
                        lhsT=kT_sb[base:base + 64, hp, P * j:P * (j + 1)],
                        rhs=qT_sb[base:base + 64, hp, t0 + o:t0 + e],
                        start=True, stop=not (diag and pi == 0))
                if diag:  # causal mask: accumulate -1e4 below the diagonal
                    nc.tensor.matmul(
                        sc[:, off:off + P], lhsT=mey_sb, rhs=mls_sb,
                        start=False, stop=True)
                wt = wt_pool.tile([P, THALF], b16, tag="wt", name=f"wt{h}_{th}_{j}")
                nc.scalar.activation(out=wt[:, off:THALF], in_=sc[:, off:THALF],
                                     func=EXP, scale=SCALE)
                if pend is not None:
                    emit_av(pend)
                if mid is not None and j in mid:
                    mid[j]()
                pend = (j, pieces, wt)
            emit_av(pend)
            # evacuate the accumulator in one fast copy (frees the PSUM
            # slot for the next unit), then normalize u/Z off-path from SBUF
            avc = norm_pool.tile([D, THALF], f32, tag="avc", name=f"avc{h}_{th}")
            nc.vector.tensor_copy(out=avc, in_=av[0:D, 0:THALF])
            # custom-DVE recip needs its input at base partition 0; ACT is
            # idle at unit boundaries, so it takes the Z extraction
            zrow = norm_pool.tile([1, THALF], f32, tag="zrow", name=f"zr{h}_{th}")
            nc.scalar.activation(out=zrow, in_=av[D:D + 1, 0:THALF],
                                 func=mybir.ActivationFunctionType.Copy)
            zr = norm_pool.tile([1, THALF], f32, tag="zr", name=f"zrr{h}_{th}")
            nc.vector.reciprocal_approx_fast(out=zr, in_=zrow)
            zb = norm_pool.tile([64, THALF], f32, tag="zb", name=f"zb{h}_{th}")
            zr_b = bass.AP(tensor=zr.tensor, offset=zr.offset,
                           ap=[list(zr.ap[0]), [0, 64], [1, THALF]])
            nc.sync.dma_start(out=zb, in_=zr_b)
            stage = norm_pool.tile([64, THALF], b16, tag="stage", name=f"st{h}_{th}")
            nc.vector.tensor_mul(out=stage, in0=avc[0:64, 0:THALF], in1=zb)
            nc.sync.dma_start(out=cc_in[th][64 * h:64 * (h + 1), :], in_=stage)

        RG = [[0, 1], [2, 3], [4, 5], [6, 7]]

        def allgather(th, p):
          # head pair {2p, 2p+1} of token-half th -> ci-tiles p (rank0) and
          # 4+p (rank1), each complete
          with nc.named_scope(f"ag{th}_{p}"):
            import concourse.mybir as mybir_mod
            nc.gpsimd.collective_compute(
                "AllGather", mybir_mod.AluOpType.bypass, replica_groups=RG,
                ins=[cc_in[th][128 * p:128 * (p + 1), :].opt()],
                outs=[cc_out[th][p].opt()])
            cc_r = cc_out[th][p].rearrange("(ci p2) t -> ci p2 t", p2=P)
            nc.sync.dma_start(out=ccout_sb[:, th, p, :], in_=cc_r[0])
            nc.sync.dma_start(out=ccout_sb[:, th, 4 + p, :], in_=cc_r[1])

        def ffn_tile0(tt):
          # full single-pass FFN tile for token-half 0 (all AGs landed)
          with nc.named_scope("ffn"):
            ps = flex_pool.tile([P, COH], f32, tag="flex", name=f"fps{tt}")
            for k, ci in enumerate((0, 4, 1, 5, 2, 6, 3, 7)):
                nc.tensor.matmul(
                    ps, lhsT=ccout_sb[:, 0, ci, P * tt:P * (tt + 1)],
                    rhs=wfT_sb[:, ci, :],
                    start=(k == 0), stop=(k == NCT - 1))
            ysb = y_pool.tile([P, COH], f32, tag="y", name=f"y{tt}")
            nc.vector.tensor_add(out=ysb, in0=ps, in1=biasb_sb)
            nc.vector.tensor_scalar_max(ysb, ysb, 0.0)
            nc.sync.dma_start(out=y.rearrange("(tt p) co -> tt p co", p=P)[tt],
                              in_=ysb)

        # ---- filler queues: projection/FFN chunks drained into the
        # attention j-loops to keep TensorE dense (and the HAM gate warm).
        fillQA = deque()   # qk projection chunks (input-DMA gated only)
        fillQB = deque()   # th0 FFN tiles (gated on the th0 AllGathers)
        for hp in (1, 2, 3):
            for i in (0, 4, 1, 5):
                fillQA.append(lambda hp=hp, i=i: qk_chunk(hp, i))
        for hp in (0, 1, 2, 3):
            for i in (2, 6, 3, 7):
                fillQA.append(lambda hp=hp, i=i: qk_chunk(hp, i))
        for tt in range(8):
            fillQB.append(lambda tt=tt: ffn_tile0(tt))

        def popA():
            if fillQA:
                fillQA.popleft()()

        def popB():
            if fillQA:
                fillQA.popleft()()
            elif fillQB:
                fillQB.popleft()()

        # ---- emission order --------------------------------------------------
        # upfront: q/k for head-pair 0 over tokens 0:1024, v tiles 0:4
        for i in (0, 4, 1, 5):
            qk_chunk(0, i)
        for st in range(4):
            v_proj(st)
        # token-half 0 attention; v st4-7 finish inside unit 0
        attn_unit(0, 0, mid={1: lambda: v_proj(4), 2: lambda: v_proj(5),
                             3: lambda: v_proj(6), 4: lambda: v_proj(7),
                             5: popA, 7: popA})
        popA()
        for h in range(1, HPC):
            attn_unit(h, 0, mid={1: popA, 3: popA, 5: popA, 7: popA})
            if h % 2 == 1:
                allgather(0, h // 2)
            popA()
            popA()
        while fillQA:  # qk work must finish before token-half 1 starts
            popA()

        # token-half 1 attention; v s-tiles 8..15 interleave inside the first
        # unit (needed from its j=8 on); th0 FFN tiles fill later units
        attn_unit(0, 1, mid={j: (lambda st=8 + j: v_proj(st)) for j in range(8)})
        allgather_last_emitted = False
        for h in range(1, HPC):
            mids = {} if h == HPC - 1 else {3: popB, 9: popB}
            attn_unit(h, 1, mid=mids)
            if h % 2 == 1 and h < HPC - 1:
                allgather(1, h // 2)
            if h < HPC - 1:
                popB()
        allgather(1, 3)
        while fillQB:
            fillQB.popleft()()

        # ---- token-half 1 FFN in three phases: ci{0,1,4,5} (pairs 0,1 landed
        # long ago), ci{2,6} (pair 2), then only ci{3,7} waits on the last AG.
        with nc.named_scope("ffn1"):
            ftiles = []
            for bi in range(2):
                buf = sc_pool.tile([P, 2 * COH], f32, tag="sc", name=f"fpsc{bi}")
                ftiles += [buf[:, 0:COH], buf[:, COH:2 * COH]]
            buf = av_pool.tile([P, 2 * COH], f32, tag="av", name="fpav")
            ftiles += [buf[:, 0:COH], buf[:, COH:2 * COH]]
            ftiles += [flex_pool.tile([P, COH], f32, tag="flex", name=f"fpfx{i}")
                       for i in range(2)]
            for phase in ((0, 4, 1, 5), (2, 6), (3, 7)):
                for tl in range(8):
                    for ci in phase:
                        nc.tensor.matmul(
                            ftiles[tl], lhsT=ccout_sb[:, 1, ci, P * tl:P * (tl + 1)],
                            rhs=wfT_sb[:, ci, :],
                            start=(ci == 0), stop=(ci == 7))
            for tl in range(8):
                ysb = y_pool.tile([P, COH], f32, tag="y", name=f"y1_{tl}")
                nc.vector.tensor_add(out=ysb, in0=ftiles[tl], in1=biasb_sb)
                nc.vector.tensor_scalar_max(ysb, ysb, 0.0)
                nc.sync.dma_start(
                    out=y.rearrange("(tt p) co -> tt p co", p=P)[8 + tl],
                    in_=ysb)

    nc.compile()
    return nc


def make_in_maps(x, Wq, Wk, Wv, Wf, bf):
    x = np.asarray(x, np.float32)
    mey_m = np.ascontiguousarray(-10000.0 * np.eye(P, dtype=np.float32)).astype(bf16)
    mls_m = np.ascontiguousarray(
        np.tril(np.ones((P, P), np.float32), -1)).astype(bf16)
    bf_f = np.asarray(bf, np.float32)
    wfT_f = np.asarray(Wf, np.float32).T
    in_maps = []
    for core in range(8):
        b, p = divmod(core, 2)
        sl = slice(HPC * p, HPC * (p + 1))
        in_maps.append({
            "xT": np.ascontiguousarray(x[b].T).astype(bf16),
            "wq": np.ascontiguousarray(
                np.asarray(Wq, np.float32)[:, sl].reshape(C, HPC * D)).astype(bf16),
            "wk": np.ascontiguousarray(
                np.asarray(Wk, np.float32)[:, sl].reshape(C, HPC * D)).astype(bf16),
            "wv": np.ascontiguousarray(
                np.asarray(Wv, np.float32)[:, sl].reshape(C, HPC * D)).astype(bf16),
            "wfT": np.ascontiguousarray(
                wfT_f[:, COH * p:COH * (p + 1)]).astype(bf16),
            "mey": mey_m,
            "mls": mls_m,
            "biasb": np.ascontiguousarray(np.tile(
                bf_f[None, COH * p:COH * (p + 1)], (P, 1))),
        })
    return in_maps


def run(x, Wq, Wk, Wv, Wf, bf, trace=False, **spmd_kwargs):
    from concourse.bass_utils import run_bass_kernel_spmd

    if "nc" not in _CACHE:
        _CACHE["nc"] = build_nc()
    nc = _CACHE["nc"]
    in_maps = make_in_maps(x, Wq, Wk, Wv, Wf, bf)
    res = run_bass_kernel_spmd(
        nc, in_maps, core_ids=list(range(8)), trace=trace, **spmd_kwargs)
    out = np.zeros((B, T, C), np.float32)
    for core in range(8):
        b, p = divmod(core, 2)
        out[b, :, COH * p:COH * (p + 1)] = res.results[core]["y"]
    return out, res


def kernel(x, Wq, Wk, Wv, Wf, bf):
    out, _ = run(x, Wq, Wk, Wv, Wf, bf, trace=False)
    return out


# revision 13
# speedup vs baseline: 1.1683x; 1.1683x over previous
"""Trainium2 Bass kernel for a dense transformer block (attention + ReLU FFN).

Reference computation (B=4, T=2048, C=1024, H=16, D=64):
    q,k,v = per-head projections of x;  causal softmax(q k^T / sqrt(C)) v;
    concat heads;  y = relu(out @ Wf.T + bf)

Sharding over 8 NeuronCores: core (2b+p) handles batch b with heads
[8p, 8p+8).  Attention runs causally over the full T on each core.  Pair
AllGathers (cores 2b/2b+1) share the attention outputs, and each core
runs the FFN for all 2048 tokens over its own half of the output
channels (the channel split is carried entirely by per-core input data -
every core executes an identical NEFF).

Layouts: scores are computed transposed ([s, t], keys on partitions) so
the exp() output feeds the AV matmul directly; V carries an appended
ones-column so row 64 of the AV accumulator is the softmax denominator;
causal masking is a -1e4 rank-128 matmul accumulated into the diagonal
score tile before exp. Compute dtype bf16 with fp32 PSUM accumulation.

Scheduling: a 48-matmul warmup burst on the (tiny, loaded-first) mask
constants heats the PE HAM clock-gate while the input DMAs stream;
projection/FFN matmul chunks are drained from filler queues inside the
attention j-loops so the PE never idles long enough to re-throttle; the
AllGathers run pair-wise and are emitted as soon as their two heads are
staged (the z-broadcast DMAs ride the Sync queue so the GpSimd queue
holds only collective triggers); the second-half FFN accumulates in
three phases so only the ci{3,7} matmuls depend on the last AllGather.
"""

import os
import sys

from collections import deque

import numpy as np
import ml_dtypes

for _p in ("/opt/trn_rl_repo", "/root/.axon_site/_ro/trn_rl_repo"):
    if os.path.isdir(_p) and _p not in sys.path:
        sys.path.append(_p)

B, T, C, H, D = 4, 2048, 1024, 16, 64
P = 128           # partitions
NCT = C // P      # 8 c-tiles
NTT = T // P      # 16 s/t-tiles
HPC = H // 2      # 8 heads per core
THALF = T // 2    # tokens per AllGather half
COH = C // 2      # output channels per core in the FFN
SCALE = float(C) ** -0.5
WARM_N = 48       # PE warmup matmuls (heats the HAM clock gate)

bf16 = ml_dtypes.bfloat16

_CACHE = {}


def build_nc():
    import concourse.bass as bass
    import concourse.tile as tile
    from concourse import bacc, mybir

    f32 = mybir.dt.float32
    b16 = mybir.dt.bfloat16
    EXP = mybir.ActivationFunctionType.Exp

    nc = bacc.Bacc("TRN2", target_bir_lowering=False, debug=False, num_devices=8)

    xT = nc.dram_tensor("xT", [C, T], b16, kind="ExternalInput").ap()
    wq = nc.dram_tensor("wq", [C, HPC * D], b16, kind="ExternalInput").ap()
    wk = nc.dram_tensor("wk", [C, HPC * D], b16, kind="ExternalInput").ap()
    wv = nc.dram_tensor("wv", [C, HPC * D], b16, kind="ExternalInput").ap()
    wfT = nc.dram_tensor("wfT", [C, COH], b16, kind="ExternalInput").ap()
    mey = nc.dram_tensor("mey", [P, P], b16, kind="ExternalInput").ap()
    mls_ = nc.dram_tensor("mls", [P, P], b16, kind="ExternalInput").ap()
    biasb = nc.dram_tensor("biasb", [P, COH], f32, kind="ExternalInput").ap()
    y = nc.dram_tensor("y", [T, COH], b16, kind="ExternalOutput").ap()
    # warmup sink: ExternalOutput so the warmup matmuls can't be DCE'd
    wsink = nc.dram_tensor("wsink", [P, P], f32, kind="ExternalOutput").ap()

    with tile.TileContext(nc) as tc, \
            tc.tile_pool(name="consts", bufs=1) as consts, \
            tc.tile_pool(name="dram", bufs=1, space="DRAM") as dram, \
            tc.tile_pool(name="sc_ps", bufs=2, space="PSUM") as sc_pool, \
            tc.tile_pool(name="av_ps", bufs=1, space="PSUM") as av_pool, \
            tc.tile_pool(name="flex_ps", bufs=2, space="PSUM") as flex_pool, \
            tc.tile_pool(name="wt", bufs=3) as wt_pool, \
            tc.tile_pool(name="norm", bufs=2) as norm_pool, \
            tc.tile_pool(name="yout", bufs=3) as y_pool:

        xT_sb = consts.tile([P, NCT, T], b16)
        wq_sb = consts.tile([P, NCT, HPC * D], b16)
        wk_sb = consts.tile([P, NCT, HPC * D], b16)
        wv_sb = consts.tile([P, NCT, HPC * D], b16)
        wfT_sb = consts.tile([P, NCT, COH], b16)
        mey_sb = consts.tile([P, P], b16)
        mls_sb = consts.tile([P, P], b16)
        biasb_sb = consts.tile([P, COH], f32)
        qT_sb = consts.tile([P, HPC // 2, T], b16)
        kT_sb = consts.tile([P, HPC // 2, T], b16)
        v_sb = consts.tile([P, NTT, HPC, D + 1], b16)
        ccout_sb = consts.tile([P, 2, NCT, THALF], b16)

        cc_in = [dram.tile([HPC * D, THALF], b16, name=f"cc_in{i}") for i in (0, 1)]
        cc_out = [[dram.tile([C // 4, THALF], b16, name=f"cc_out{th}_{p}")
                   for p in range(4)] for th in (0, 1)]

        # ---- constant loads: one batched DMA per tensor (per 512-column
        # block for x), weights on the Sync queue and x on the Scalar queue
        # so the preamble isn't serialized on a single engine's DMA issue.
        nc.sync.dma_start(out=mey_sb, in_=mey)
        nc.sync.dma_start(out=mls_sb, in_=mls_)
        xT_p = xT.rearrange("(ct p) t -> p ct t", p=P)
        nc.sync.dma_start(out=wq_sb, in_=wq.rearrange("(ct p) m -> p ct m", p=P))
        nc.scalar.dma_start(out=xT_sb[:, :, 0:512], in_=xT_p[:, :, 0:512])
        nc.sync.dma_start(out=wk_sb, in_=wk.rearrange("(ct p) m -> p ct m", p=P))
        nc.scalar.dma_start(out=xT_sb[:, :, 512:THALF],
                            in_=xT_p[:, :, 512:THALF])
        nc.sync.dma_start(out=wv_sb, in_=wv.rearrange("(ct p) m -> p ct m", p=P))
        nc.scalar.dma_start(out=xT_sb[:, :, THALF:THALF + 512],
                            in_=xT_p[:, :, THALF:THALF + 512])
        nc.scalar.dma_start(out=xT_sb[:, :, THALF + 512:T],
                            in_=xT_p[:, :, THALF + 512:T])
        nc.sync.dma_start(out=wfT_sb,
                          in_=wfT.rearrange("(ct p) co -> p ct co", p=P))
        nc.sync.dma_start(out=biasb_sb, in_=biasb)
        nc.vector.memset(v_sb[:, :, :, D:D + 1], 1.0)

        # ---- PE warmup: dense matmul burst on the mask constants while the
        # big input DMAs stream; keeps the HAM gate at 8/8 for the real work.
        with nc.named_scope("warmup"):
            wps = flex_pool.tile([P, P], f32, tag="flex", name="warmps")
            for i in range(WARM_N):
                nc.tensor.matmul(wps, lhsT=mey_sb, rhs=mls_sb,
                                 start=(i == 0), stop=(i == WARM_N - 1))
            wsb = y_pool.tile([P, P], f32, tag="y", name="warmsb")
            nc.vector.tensor_copy(out=wsb, in_=wps)
            nc.sync.dma_start(out=wsink, in_=wsb)

        # ---- emission helpers ----------------------------------------------
        def v_proj(st):
          with nc.named_scope("vproj"):
            ps = flex_pool.tile([P, 512], f32, tag="flex", name=f"vps{st}")
            for ct in range(NCT):
                nc.tensor.matmul(
                    ps, lhsT=xT_sb[:, ct, P * st:P * (st + 1)],
                    rhs=wv_sb[:, ct, :],
                    start=(ct == 0), stop=(ct == NCT - 1))
            nc.vector.tensor_copy(out=v_sb[:, st, :, 0:D],
                                  in_=ps.rearrange("p (h d) -> p h d", d=D))

        def qk_chunk(hp, i):
          with nc.named_scope("qkproj"):
            dst, w_t = ((qT_sb, wq_sb), (kT_sb, wk_sb))[i // 4]
            g = i % 4
            ps = flex_pool.tile([P, 512], f32, tag="flex", name=f"qkps{hp}_{i}")
            for ct in range(NCT):
                nc.tensor.matmul(
                    ps, lhsT=w_t[:, ct, hp * P:(hp + 1) * P],
                    rhs=xT_sb[:, ct, 512 * g:512 * (g + 1)],
                    start=(ct == 0), stop=(ct == NCT - 1))
            nc.vector.tensor_copy(out=dst[:, hp, 512 * g:512 * (g + 1)], in_=ps)

        def attn_unit(h, th, mid=None):
          with nc.named_scope(f"attn{th}_{h}"):
            hp, qh = divmod(h, 2)
            base = 64 * qh
            t0 = THALF * th
            av = av_pool.tile([P, THALF], f32, tag="av", name=f"av{h}_{th}")
            jmax = 8 * th + 8
            last_j = {0: 8 * th + 3, 1: jmax - 1}
            pend = None  # (j, pieces, wt) awaiting its AV emission

            def emit_av(ent):
                j, pieces, wt = ent
                for (o, e) in pieces:
                    region = 0 if o < 512 else 1
                    nc.tensor.matmul(
                        av[0:D + 1, o:e], lhsT=v_sb[:, j, h, :], rhs=wt[:, o:e],
                        start=(j == 0), stop=(j == last_j[region]))

            for j in range(jmax):
                off = max(0, P * j - t0)
                diag = P * j >= t0
                pieces = [(off, 512), (512, 1024)] if off < 512 \
                    else [(off, 1024)]
                sc = sc_pool.tile([P, THALF], f32, tag="sc", name=f"sc{h}_{th}_{j}")
                for pi, (o, e) in enumerate(pieces):
                    nc.tensor.matmul(
                        sc[:, o:e],
                        lhsT=kT_sb[base:base + 64, hp, P * j:P * (j + 1)],
                        rhs=qT_sb[base:base + 64, hp, t0 + o:t0 + e],
                        start=True, stop=not (diag and pi == 0))
                if diag:  # causal mask: accumulate -1e4 below the diagonal
                    nc.tensor.matmul(
                        sc[:, off:off + P], lhsT=mey_sb, rhs=mls_sb,
                        start=False, stop=True)
                wt = wt_pool.tile([P, THALF], b16, tag="wt", name=f"wt{h}_{th}_{j}")
                nc.scalar.activation(out=wt[:, off:THALF], in_=sc[:, off:THALF],
                                     func=EXP, scale=SCALE)
                if pend is not None:
                    emit_av(pend)
                if mid is not None and j in mid:
                    mid[j]()
                pend = (j, pieces, wt)
            emit_av(pend)
            # evacuate the accumulator in one fast copy (frees the PSUM
            # slot for the next unit), then normalize u/Z off-path from SBUF
            avc = norm_pool.tile([D, THALF], f32, tag="avc", name=f"avc{h}_{th}")
            nc.vector.tensor_copy(out=avc, in_=av[0:D, 0:THALF])
            # custom-DVE recip needs its input at base partition 0; ACT is
            # idle at unit boundaries, so it takes the Z extraction
            zrow = norm_pool.tile([1, THALF], f32, tag="zrow", name=f"zr{h}_{th}")
            nc.scalar.activation(out=zrow, in_=av[D:D + 1, 0:THALF],
                                 func=mybir.ActivationFunctionType.Copy)
            zr = norm_pool.tile([1, THALF], f32, tag="zr", name=f"zrr{h}_{th}")
            nc.vector.reciprocal_approx_fast(out=zr, in_=zrow)
            zb = norm_pool.tile([64, THALF], f32, tag="zb", name=f"zb{h}_{th}")
            zr_b = bass.AP(tensor=zr.tensor, offset=zr.offset,
                           ap=[list(zr.ap[0]), [0, 64], [1, THALF]])
            # zb/stage ride the GpSimd queue: they are produced at unit tails,
            # so putting them on Sync would make every later consumer of a
            # Sync-queue DMA transitively wait on this unit's normalize chain.
            nc.gpsimd.dma_start(out=zb, in_=zr_b)
            stage = norm_pool.tile([64, THALF], b16, tag="stage", name=f"st{h}_{th}")
            nc.vector.tensor_mul(out=stage, in0=avc[0:64, 0:THALF], in1=zb)
            nc.gpsimd.dma_start(out=cc_in[th][64 * h:64 * (h + 1), :], in_=stage)

        RG = [[0, 1], [2, 3], [4, 5], [6, 7]]

        def allgather(th, p):
          # head pair {2p, 2p+1} of token-half th -> ci-tiles p (rank0) and
          # 4+p (rank1), each complete
          with nc.named_scope(f"ag{th}_{p}"):
            import concourse.mybir as mybir_mod
            nc.gpsimd.collective_compute(
                "AllGather", mybir_mod.AluOpType.bypass, replica_groups=RG,
                ins=[cc_in[th][128 * p:128 * (p + 1), :].opt()],
                outs=[cc_out[th][p].opt()])
            cc_r = cc_out[th][p].rearrange("(ci p2) t -> ci p2 t", p2=P)
            nc.sync.dma_start(out=ccout_sb[:, th, p, :], in_=cc_r[0])
            nc.sync.dma_start(out=ccout_sb[:, th, 4 + p, :], in_=cc_r[1])

        def ffn_tile0(tt):
          # full single-pass FFN tile for token-half 0 (all AGs landed)
          with nc.named_scope("ffn"):
            ps = flex_pool.tile([P, COH], f32, tag="flex", name=f"fps{tt}")
            for k, ci in enumerate((0, 4, 1, 5, 2, 6, 3, 7)):
                nc.tensor.matmul(
                    ps, lhsT=ccout_sb[:, 0, ci, P * tt:P * (tt + 1)],
                    rhs=wfT_sb[:, ci, :],
                    start=(k == 0), stop=(k == NCT - 1))
            ysb = y_pool.tile([P, COH], b16, tag="y", name=f"y{tt}")
            nc.vector.tensor_add(out=ysb, in0=ps, in1=biasb_sb)
            nc.vector.tensor_scalar_max(ysb, ysb, 0.0)
            nc.sync.dma_start(out=y.rearrange("(tt p) co -> tt p co", p=P)[tt],
                              in_=ysb)

        # ---- filler queues: projection/FFN chunks drained into the
        # attention j-loops to keep TensorE dense (and the HAM gate warm).
        fillQA = deque()   # qk projection chunks (input-DMA gated only)
        fillQB = deque()   # th0 FFN tiles (gated on the th0 AllGathers)
        for hp in (1, 2, 3):
            for i in (0, 4, 1, 5):
                fillQA.append(lambda hp=hp, i=i: qk_chunk(hp, i))
        for hp in (0, 1, 2, 3):
            for i in (2, 6, 3, 7):
                fillQA.append(lambda hp=hp, i=i: qk_chunk(hp, i))
        for tt in range(8):
            fillQB.append(lambda tt=tt: ffn_tile0(tt))

        def popA():
            if fillQA:
                fillQA.popleft()()

        def popB():
            if fillQA:
                fillQA.popleft()()
            elif fillQB:
                fillQB.popleft()()

        # ---- emission order --------------------------------------------------
        # upfront: q/k for head-pair 0 over tokens 0:1024, v tiles 0:4
        for i in (0, 4, 1, 5):
            qk_chunk(0, i)
        for st in range(4):
            v_proj(st)
        # token-half 0 attention; v st4-7 finish inside unit 0
        attn_unit(0, 0, mid={1: lambda: v_proj(4), 2: lambda: v_proj(5),
                             3: lambda: v_proj(6), 4: lambda: v_proj(7),
                             6: popA})
        popA()
        for h in range(1, HPC):
            attn_unit(h, 0, mid={1: popA, 3: popA})
            if h % 2 == 1:
                allgather(0, h // 2)
            popA()

        # token-half 1 attention; v s-tiles 8..15 interleave inside the first
        # unit (needed from its j=8 on); th0 FFN tiles fill later units
        attn_unit(0, 1, mid={j: (lambda st=8 + j: v_proj(st)) for j in range(8)})
        for h in range(1, HPC):
            mids = {2: popB, 6: popB} if h == HPC - 1 else {3: popB, 9: popB}
            attn_unit(h, 1, mid=mids)
            if h % 2 == 1 and h < HPC - 1:
                allgather(1, h // 2)
            if h < HPC - 1:
                popB()
        allgather(1, 3)
        while fillQA or fillQB:
            popB()

        # ---- token-half 1 FFN in three phases: ci{0,1,4,5} (pairs 0,1 landed
        # long ago), ci{2,6} (pair 2), then only ci{3,7} waits on the last AG.
        with nc.named_scope("ffn1"):
            ftiles = []
            for bi in range(2):
                buf = sc_pool.tile([P, 2 * COH], f32, tag="sc", name=f"fpsc{bi}")
                ftiles += [buf[:, 0:COH], buf[:, COH:2 * COH]]
            buf = av_pool.tile([P, 2 * COH], f32, tag="av", name="fpav")
            ftiles += [buf[:, 0:COH], buf[:, COH:2 * COH]]
            ftiles += [flex_pool.tile([P, COH], f32, tag="flex", name=f"fpfx{i}")
                       for i in range(2)]
            for phase in ((0, 4, 1, 5), (2, 6), (3, 7)):
                for tl in range(8):
                    for ci in phase:
                        nc.tensor.matmul(
                            ftiles[tl], lhsT=ccout_sb[:, 1, ci, P * tl:P * (tl + 1)],
                            rhs=wfT_sb[:, ci, :],
                            start=(ci == 0), stop=(ci == 7))
            for tl in range(8):
                ysb = y_pool.tile([P, COH], b16, tag="y", name=f"y1_{tl}")
                nc.vector.tensor_add(out=ysb, in0=ftiles[tl], in1=biasb_sb)
                nc.vector.tensor_scalar_max(ysb, ysb, 0.0)
                nc.sync.dma_start(
                    out=y.rearrange("(tt p) co -> tt p co", p=P)[8 + tl],
                    in_=ysb)

    nc.compile()
    return nc


def make_in_maps(x, Wq, Wk, Wv, Wf, bf):
    x = np.asarray(x, np.float32)
    mey_m = np.ascontiguousarray(-10000.0 * np.eye(P, dtype=np.float32)).astype(bf16)
    mls_m = np.ascontiguousarray(
        np.tril(np.ones((P, P), np.float32), -1)).astype(bf16)
    bf_f = np.asarray(bf, np.float32)
    wfT_f = np.asarray(Wf, np.float32).T
    in_maps = []
    for core in range(8):
        b, p = divmod(core, 2)
        sl = slice(HPC * p, HPC * (p + 1))
        in_maps.append({
            "xT": np.ascontiguousarray(x[b].T).astype(bf16),
            "wq": np.ascontiguousarray(
                np.asarray(Wq, np.float32)[:, sl].reshape(C, HPC * D)).astype(bf16),
            "wk": np.ascontiguousarray(
                np.asarray(Wk, np.float32)[:, sl].reshape(C, HPC * D)).astype(bf16),
            "wv": np.ascontiguousarray(
                np.asarray(Wv, np.float32)[:, sl].reshape(C, HPC * D)).astype(bf16),
            "wfT": np.ascontiguousarray(
                wfT_f[:, COH * p:COH * (p + 1)]).astype(bf16),
            "mey": mey_m,
            "mls": mls_m,
            "biasb": np.ascontiguousarray(np.tile(
                bf_f[None, COH * p:COH * (p + 1)], (P, 1))),
        })
    return in_maps


def run(x, Wq, Wk, Wv, Wf, bf, trace=False, **spmd_kwargs):
    from concourse.bass_utils import run_bass_kernel_spmd

    if "nc" not in _CACHE:
        _CACHE["nc"] = build_nc()
    nc = _CACHE["nc"]
    in_maps = make_in_maps(x, Wq, Wk, Wv, Wf, bf)
    res = run_bass_kernel_spmd(
        nc, in_maps, core_ids=list(range(8)), trace=trace, **spmd_kwargs)
    out = np.zeros((B, T, C), np.float32)
    for core in range(8):
        b, p = divmod(core, 2)
        out[b, :, COH * p:COH * (p + 1)] = \
            np.asarray(res.results[core]["y"]).astype(np.float32)
    return out, res


def kernel(x, Wq, Wk, Wv, Wf, bf):
    out, _ = run(x, Wq, Wk, Wv, Wf, bf, trace=False)
    return out
